# revision 5
# baseline (speedup 1.0000x reference)
"""GroupedQueryAttention Bass kernel for 8 TRN2 NeuronCores.

Sharding: core c handles batch b = c//4 and query-row slice s = c%4
(1024 of 4096 rows). Each core computes full K/V for its batch
(redundant across the 4 cores of a batch group) and the final output
rows for its (b, s) slice — no collectives needed.

Per-core math (all matmuls bf16, fp32 PSUM accumulation):
  K^T, V, Q^T projections directly in transposed layout
  RoPE applied on [head_dim, n] layout via partition-block swaps
  S^T = K_g^T-chunks.T @ Q_h^T   (contraction over head_dim=64)
  P^T = exp(S^T / 8)             (no max subtraction; scores are small)
  O^T[65] = [V_g | 1]^T @ P^T    (row 64 = softmax denominators)
  Ot = O^T[0:64] * (1/denominator)  (DMA-replicated broadcast)
  Y = Ot.T @ Wo                  (row-parallel over full Wo)
"""

import numpy as np
from ml_dtypes import bfloat16

import concourse.bass as bass
import concourse.mybir as mybir
from concourse import bacc, tile, bass_utils

F32 = mybir.dt.float32
BF16 = mybir.dt.bfloat16
EXP = mybir.ActivationFunctionType.Exp

B, N, D = 2, 4096, 1024
HQ, HKV, HD = 16, 4, 64
G = HQ // HKV          # 4 q heads per kv head
NQ = N // 4            # 1024 q rows per core
N_CORES = 8

_CACHE = {}


def _build():
    nc = bacc.Bacc("TRN2", target_bir_lowering=False, debug=False,
                   num_devices=N_CORES)

    xT_d = nc.dram_tensor("xT", [D, N], BF16, kind="ExternalInput").ap()
    xqT_d = nc.dram_tensor("xqT", [D, NQ], BF16, kind="ExternalInput").ap()
    Wq_d = nc.dram_tensor("Wq", [D, D], BF16, kind="ExternalInput").ap()
    Wk_d = nc.dram_tensor("Wk", [D, 256], BF16, kind="ExternalInput").ap()
    Wv_d = nc.dram_tensor("Wv", [D, 256], BF16, kind="ExternalInput").ap()
    Wo_d = nc.dram_tensor("Wo", [D, D], BF16, kind="ExternalInput").ap()
    cosK_d = nc.dram_tensor("cosK", [128, N], F32, kind="ExternalInput").ap()
    nsinK_d = nc.dram_tensor("nsinK", [128, N], F32, kind="ExternalInput").ap()
    cosQ_d = nc.dram_tensor("cosQ", [128, NQ], F32, kind="ExternalInput").ap()
    nsinQ_d = nc.dram_tensor("nsinQ", [128, NQ], F32, kind="ExternalInput").ap()
    out_d = nc.dram_tensor("out", [NQ, D], F32, kind="ExternalOutput").ap()

    NKC = N // 128          # 32 key chunks
    VBLK = 65               # V block width: 64 V cols + ones col

    with tile.TileContext(nc) as tc:
        with tc.tile_pool(name="persist", bufs=1) as pp, \
             tc.tile_pool(name="dram", bufs=1, space="DRAM") as dp:

            # ---- persistent SBUF tensors ----
            Qrt = [pp.tile([128, 4 * NQ], BF16, tag=f"qrt{t}", name=f"qrt{t}")
                   for t in range(2)]
            Krt = [pp.tile([128, N], BF16, tag=f"krt{t}", name=f"krt{t}")
                   for t in range(2)]
            Vsb = pp.tile([128, NKC * HKV * VBLK], BF16, tag="vsb", name="vsb")
            Ot = [pp.tile([128, NQ], BF16, tag=f"ot{i}", name=f"ot{i}")
                  for i in range(8)]
            Wo16 = [pp.tile([128, D], BF16, tag=f"wo{i}", name=f"wo{i}")
                    for i in range(8)]
            for i in range(8):
                nc.sync.dma_start(Wo16[i][:], Wo_d[i * 128:(i + 1) * 128, :])

            # ones column in every V block
            ones_c = pp.tile([128, 1], BF16, tag="ones", name="ones")
            nc.vector.memset(ones_c[:], 1.0)
            vview = Vsb.rearrange("p (b c) -> p b c", c=VBLK)[:, :, 64:65]
            ones_b = bass.AP(ones_c.tensor, ones_c.offset,
                             [ones_c.ap[0], [0, NKC * HKV], [1, 1]])
            nc.vector.tensor_copy(vview, ones_b)

            # ================= phase A: projections + rope ================
            with tc.tile_pool(name="pa", bufs=1) as pa, \
                 tc.tile_pool(name="pap", bufs=1, space="PSUM") as pap:

                Wks = pa.tile([128, 8 * 256], BF16, tag="wks", name="wks")
                Wvs = pa.tile([128, 8 * 256], BF16, tag="wvs", name="wvs")
                for dc in range(8):
                    nc.sync.dma_start(Wks[:, dc * 256:(dc + 1) * 256],
                                      Wk_d[dc * 128:(dc + 1) * 128, :])
                    nc.sync.dma_start(Wvs[:, dc * 256:(dc + 1) * 256],
                                      Wv_d[dc * 128:(dc + 1) * 128, :])
                cosK = pa.tile([128, N], F32, tag="cosk", name="cosk")
                nsinK = pa.tile([128, N], F32, tag="nsink", name="nsink")
                cosQ = pa.tile([128, NQ], F32, tag="cosq", name="cosq")
                nsinQ = pa.tile([128, NQ], F32, tag="nsinq", name="nsinq")
                nc.sync.dma_start(cosK[:], cosK_d[:])
                nc.sync.dma_start(nsinK[:], nsinK_d[:])
                nc.sync.dma_start(cosQ[:], cosQ_d[:])
                nc.sync.dma_start(nsinQ[:], nsinQ_d[:])

                def rope(psum, cos_s, nsin_s, w):
                    """Return (t1, tmp) f32 tiles; result = t1 + tmp (caller adds)."""
                    tmp = pa.tile([128, w], F32, tag="rtmp", bufs=3, name="rtmp")
                    for blk in range(4):
                        src = (blk ^ 1) * 32
                        nc.vector.tensor_copy(tmp[blk * 32:(blk + 1) * 32, :],
                                              psum[src:src + 32, :])
                    t1 = pa.tile([128, w], F32, tag="rt1", bufs=3, name="rt1")
                    nc.vector.tensor_mul(t1[:], psum[:], cos_s)
                    nc.vector.tensor_mul(tmp[:], tmp[:], nsin_s)
                    return t1, tmp

                for nh in range(2):       # stream x^T in two n-halves
                    xt = []
                    for dc in range(8):
                        t = pa.tile([128, N // 2], BF16, tag=f"xt{dc}", bufs=1,
                                    name=f"xt{dc}")
                        nc.sync.dma_start(
                            t[:], xT_d[dc * 128:(dc + 1) * 128,
                                       nh * (N // 2):(nh + 1) * (N // 2)])
                        xt.append(t)

                    # V projection: [128 n, 256] psum chunks
                    for nch in range(16):
                        pv = pap.tile([128, 256], F32, tag="pv", bufs=2,
                                      name="pv")
                        for dc in range(8):
                            nc.tensor.matmul(
                                pv[:],
                                xt[dc][:, nch * 128:(nch + 1) * 128],
                                Wvs[:, dc * 256:(dc + 1) * 256],
                                start=(dc == 0), stop=(dc == 7))
                        blk0 = (nh * 16 + nch) * HKV * VBLK
                        dst = Vsb[:, blk0:blk0 + HKV * VBLK].rearrange(
                            "p (g c) -> p g c", c=VBLK)[:, :, 0:64]
                        src = pv.rearrange("p (g c) -> p g c", c=64)
                        nc.vector.tensor_copy(dst, src)

                    # K^T projection + rope: per pair tensor, [128, 512] chunks
                    for pt in range(2):
                        for nch in range(4):
                            pk = pap.tile([128, 512], F32, tag="pk", bufs=2,
                                          name="pk")
                            for dc in range(8):
                                nc.tensor.matmul(
                                    pk[:],
                                    Wks[:, dc * 256 + pt * 128:
                                        dc * 256 + (pt + 1) * 128],
                                    xt[dc][:, nch * 512:(nch + 1) * 512],
                                    start=(dc == 0), stop=(dc == 7))
                            c0 = nh * (N // 2) + nch * 512
                            t1, tmp = rope(pk, cosK[:, c0:c0 + 512],
                                           nsinK[:, c0:c0 + 512], 512)
                            nc.vector.tensor_add(Krt[pt][:, c0:c0 + 512],
                                                 t1[:], tmp[:])

                # Q^T projection + rope (uses xqT, all 1024 q rows)
                xq = []
                for dc in range(8):
                    t = pa.tile([128, NQ], BF16, tag=f"xq{dc}", name=f"xq{dc}")
                    nc.sync.dma_start(t[:], xqT_d[dc * 128:(dc + 1) * 128, :])
                    xq.append(t)
                for hp in range(8):        # head pair (2hp, 2hp+1)
                    g = hp // 2
                    t = g // 2
                    base = (g % 2) * 64
                    hi = 2 * hp - 4 * g
                    wq = pa.tile([128, 8 * 128], BF16, tag="wq", bufs=2,
                                 name="wq")
                    for dc in range(8):
                        nc.sync.dma_start(
                            wq[:, dc * 128:(dc + 1) * 128],
                            Wq_d[dc * 128:(dc + 1) * 128,
                                 hp * 128:(hp + 1) * 128])
                    for nch in range(2):
                        pq = pap.tile([128, 512], F32, tag="pk", bufs=2,
                                      name="pq")
                        for dc in range(8):
                            nc.tensor.matmul(
                                pq[:],
                                wq[:, dc * 128:(dc + 1) * 128],
                                xq[dc][:, nch * 512:(nch + 1) * 512],
                                start=(dc == 0), stop=(dc == 7))
                        c0 = nch * 512
                        t1, tmp = rope(pq, cosQ[:, c0:c0 + 512],
                                       nsinQ[:, c0:c0 + 512], 512)
                        # split heads into aligned Qrt storage
                        d0 = hi * NQ + c0
                        d1 = (hi + 1) * NQ + c0
                        nc.vector.tensor_add(
                            Qrt[t][base:base + 64, d0:d0 + 512],
                            t1[0:64, :], tmp[0:64, :])
                        nc.vector.tensor_add(
                            Qrt[t][base:base + 64, d1:d1 + 512],
                            t1[64:128, :], tmp[64:128, :])

            # ================= phase B: attention =========================
            with tc.tile_pool(name="pb", bufs=1) as pb, \
                 tc.tile_pool(name="pbp", bufs=1, space="PSUM") as pbp:
                for pt in range(2):
                    g0, g1 = 2 * pt, 2 * pt + 1
                    for hi in range(4):
                        for qh in range(2):
                            q0 = hi * NQ + qh * 512
                            oA = pbp.tile([65, 512], F32, tag="oA", bufs=2,
                                          name="oA")
                            oB = pbp.tile([65, 512], F32, tag="oB", bufs=2,
                                          name="oB")
                            for kc in range(NKC):
                                st = pbp.tile([128, 1024], F32, tag="st",
                                              bufs=2, name="st")
                                nc.tensor.matmul(
                                    st[:, 0:512],
                                    Krt[pt][0:64, kc * 128:(kc + 1) * 128],
                                    Qrt[pt][0:64, q0:q0 + 512],
                                    start=True, stop=True)
                                nc.tensor.matmul(
                                    st[:, 512:1024],
                                    Krt[pt][64:128, kc * 128:(kc + 1) * 128],
                                    Qrt[pt][64:128, q0:q0 + 512],
                                    start=True, stop=True)
                                pT = pb.tile([128, 1024], BF16, tag="pT",
                                             bufs=3, name="pT")
                                nc.scalar.activation(pT[:], st[:], EXP,
                                                     scale=0.125)
                                vb0 = (kc * HKV + g0) * VBLK
                                vb1 = (kc * HKV + g1) * VBLK
                                nc.tensor.matmul(
                                    oA[:], Vsb[:, vb0:vb0 + VBLK],
                                    pT[:, 0:512],
                                    start=(kc == 0), stop=(kc == NKC - 1))
                                nc.tensor.matmul(
                                    oB[:], Vsb[:, vb1:vb1 + VBLK],
                                    pT[:, 512:1024],
                                    start=(kc == 0), stop=(kc == NKC - 1))
                            # normalize + store to Ot
                            for o, g in ((oA, g0), (oB, g1)):
                                head = 4 * g + hi
                                hc, row = head // 2, (head % 2) * 64
                                rinv = pb.tile([1, 512], F32, tag="rinv",
                                               bufs=2, name="rinv")
                                nc.vector.reciprocal(rinv[:], o[64:65, :])
                                sc = dp.tile([1, 512], F32, tag="rsc", bufs=2,
                                             name="rsc")
                                nc.sync.dma_start(sc[:], rinv[:])
                                rb = pb.tile([64, 512], F32, tag="rb", bufs=2,
                                             name="rb")
                                nc.sync.dma_start(
                                    rb[:],
                                    bass.AP(sc.tensor, sc.offset,
                                            [[0, 64], [1, 512]]))
                                nc.vector.tensor_mul(
                                    Ot[hc][row:row + 64,
                                           qh * 512:(qh + 1) * 512],
                                    o[0:64, :], rb[:])

            # ================= phase C: output projection =================
            with tc.tile_pool(name="pc", bufs=1) as pc, \
                 tc.tile_pool(name="pcp", bufs=1, space="PSUM") as pcp:
                for qc in range(8):
                    yp = pcp.tile([128, 1024], F32, tag="y", bufs=2, name="yp")
                    for dh in range(2):
                        for hc in range(8):
                            nc.tensor.matmul(
                                yp[:, dh * 512:(dh + 1) * 512],
                                Ot[hc][:, qc * 128:(qc + 1) * 128],
                                Wo16[hc][:, dh * 512:(dh + 1) * 512],
                                start=(hc == 0), stop=(hc == 7))
                    ys = pc.tile([128, 1024], F32, tag="ys", bufs=2, name="ys")
                    nc.vector.tensor_copy(ys[:], yp[:])
                    nc.sync.dma_start(out_d[qc * 128:(qc + 1) * 128, :], ys[:])

    nc.compile()
    return nc


def get_nc():
    if "nc" not in _CACHE:
        _CACHE["nc"] = _build()
    return _CACHE["nc"]


def _rope_tables():
    inv_freq = 1.0 / (10000.0 ** (np.arange(0, HD, 2, dtype=np.float32) / HD))
    t = np.arange(N, dtype=np.float32)
    freqs = np.outer(t, inv_freq)
    emb = np.concatenate([freqs, freqs], -1)        # [N, HD]
    return np.cos(emb).astype(np.float32), np.sin(emb).astype(np.float32)


def make_in_maps(x, Wq, Wk, Wv, Wo):
    cos, sin = _rope_tables()
    cosT = np.ascontiguousarray(cos.T)              # [64, N]
    nsinT = np.ascontiguousarray(sin.T)
    nsinT[0:32] = -nsinT[0:32]
    cosK = np.vstack([cosT, cosT])                  # [128, N]
    nsinK = np.vstack([nsinT, nsinT])

    bf = lambda a: np.ascontiguousarray(a).astype(bfloat16)
    Wq16, Wk16, Wv16, Wo16 = bf(Wq), bf(Wk), bf(Wv), bf(Wo)

    in_maps = []
    for c in range(N_CORES):
        b, s = c // 4, c % 4
        xT = bf(x[b].T)                             # [D, N]
        xqT = bf(x[b, s * NQ:(s + 1) * NQ, :].T)    # [D, NQ]
        in_maps.append({
            "xT": xT, "xqT": xqT,
            "Wq": Wq16, "Wk": Wk16, "Wv": Wv16, "Wo": Wo16,
            "cosK": cosK, "nsinK": nsinK,
            "cosQ": np.ascontiguousarray(cosK[:, s * NQ:(s + 1) * NQ]),
            "nsinQ": np.ascontiguousarray(nsinK[:, s * NQ:(s + 1) * NQ]),
        })
    return in_maps


def assemble(results):
    out = np.zeros((B, N, D), np.float32)
    for c in range(N_CORES):
        b, s = c // 4, c % 4
        out[b, s * NQ:(s + 1) * NQ, :] = results[c]["out"]
    return out


def kernel(x, Wq, Wk, Wv, Wo):
    nc = get_nc()
    in_maps = make_in_maps(np.asarray(x, np.float32), np.asarray(Wq, np.float32),
                           np.asarray(Wk, np.float32), np.asarray(Wv, np.float32),
                           np.asarray(Wo, np.float32))
    res = bass_utils.run_bass_kernel_spmd(nc, in_maps,
                                          core_ids=list(range(N_CORES)))
    return assemble(res.results)


# revision 8
# speedup vs baseline: 1.1329x; 1.1329x over previous
"""GroupedQueryAttention Bass kernel for 8 TRN2 NeuronCores.

Sharding: core c handles batch b = c//4 and query-row slice s = c%4
(1024 of 4096 rows). Each core computes full K/V for its batch
(redundant across the 4 cores of a batch group) and the final output
rows for its (b, s) slice — no collectives needed.

Per-core math (all matmuls bf16, fp32 PSUM accumulation):
  K^T, V, Q^T projections directly in transposed layout
  RoPE applied on [head_dim, n] layout via partition-block swaps
  S^T = K_g^T-chunks.T @ Q_h^T   (contraction over head_dim=64)
  P^T = exp(S^T / 8)             (no max subtraction; scores are small)
  O^T[65] = [V_g | 1]^T @ P^T    (row 64 = softmax denominators)
  Ot = O^T[0:64] * (1/denominator)  (DMA-replicated broadcast)
  Y = Ot.T @ Wo                  (row-parallel over full Wo)

The attention kc-loop is split into two passes of 16 key-chunks so
pass 0 only depends on the first half of K/V — it overlaps the rest
of the projections. Pass-0 partial O accumulators round-trip DRAM.
"""

import numpy as np
from ml_dtypes import bfloat16

import concourse.bass as bass
import concourse.mybir as mybir
from concourse import bacc, tile, bass_utils

F32 = mybir.dt.float32
BF16 = mybir.dt.bfloat16
EXP = mybir.ActivationFunctionType.Exp

B, N, D = 2, 4096, 1024
HQ, HKV, HD = 16, 4, 64
G = HQ // HKV          # 4 q heads per kv head
NQ = N // 4            # 1024 q rows per core
N_CORES = 8

_CACHE = {}


def _build():
    nc = bacc.Bacc("TRN2", target_bir_lowering=False, debug=False,
                   num_devices=N_CORES)

    xT_d = nc.dram_tensor("xT", [D, N], BF16, kind="ExternalInput").ap()
    xqT_d = nc.dram_tensor("xqT", [D, NQ], BF16, kind="ExternalInput").ap()
    Wq_d = nc.dram_tensor("Wq", [D, D], BF16, kind="ExternalInput").ap()
    Wk_d = nc.dram_tensor("Wk", [D, 256], BF16, kind="ExternalInput").ap()
    Wv_d = nc.dram_tensor("Wv", [D, 256], BF16, kind="ExternalInput").ap()
    Wo_d = nc.dram_tensor("Wo", [D, D], BF16, kind="ExternalInput").ap()
    cosK_d = nc.dram_tensor("cosK", [128, N], BF16, kind="ExternalInput").ap()
    nsinK_d = nc.dram_tensor("nsinK", [128, N], BF16, kind="ExternalInput").ap()
    cosQ_d = nc.dram_tensor("cosQ", [128, NQ], BF16, kind="ExternalInput").ap()
    nsinQ_d = nc.dram_tensor("nsinQ", [128, NQ], BF16, kind="ExternalInput").ap()
    out_d = nc.dram_tensor("out", [NQ, D], F32, kind="ExternalOutput").ap()

    NKC = N // 128          # 32 key chunks
    VBLK = 65               # V block width: 64 V cols + ones col

    with tile.TileContext(nc) as tc:
      with tc.tile_pool(name="persist", bufs=1) as pp, \
           tc.tile_pool(name="dram", bufs=1, space="DRAM") as dp:
        with tc.tile_pool(name="pa", bufs=1) as pa, \
             tc.tile_pool(name="pap", bufs=1, space="PSUM") as pap, \
             tc.tile_pool(name="pb", bufs=1) as pb, \
             tc.tile_pool(name="pbp", bufs=1, space="PSUM") as pbp:

            # ---- persistent SBUF tensors ----
            Qrt = [pp.tile([128, 4 * NQ], BF16, tag=f"qrt{t}", name=f"qrt{t}")
                   for t in range(2)]
            Krt = [pp.tile([128, N], BF16, tag=f"krt{t}", name=f"krt{t}")
                   for t in range(2)]
            Vsb = pp.tile([128, NKC * HKV * VBLK], BF16, tag="vsb", name="vsb")
            Ot = [pp.tile([128, NQ], BF16, tag=f"ot{i}", name=f"ot{i}")
                  for i in range(8)]
            Wo16 = [pp.tile([128, D], BF16, tag=f"wo{i}", name=f"wo{i}")
                    for i in range(8)]

            # ---- PE warm-up: dummy matmuls while first DMAs land ----
            warm = pa.tile([128, 512], BF16, tag="warm", name="warm")
            nc.vector.memset(warm[:], 0.0)
            for w in range(16):
                wp = pap.tile([128, 512], F32, tag="mm", bufs=2, name="wp")
                nc.tensor.matmul(wp[:], warm[:, 0:128], warm[:],
                                 start=True, stop=True)

            # ones column in every V block
            ones_c = pp.tile([128, 1], BF16, tag="ones", name="ones")
            nc.vector.memset(ones_c[:], 1.0)
            vview = Vsb.rearrange("p (b c) -> p b c", c=VBLK)[:, :, 64:65]
            ones_b = bass.AP(ones_c.tensor, ones_c.offset,
                             [ones_c.ap[0], [0, NKC * HKV], [1, 1]])
            nc.vector.tensor_copy(vview, ones_b)

            # ---- input DMAs (emission order ~ priority) ----
            Wks = pa.tile([128, 8 * 256], BF16, tag="wks", name="wks")
            Wvs = pa.tile([128, 8 * 256], BF16, tag="wvs", name="wvs")
            for dc in range(8):
                nc.sync.dma_start(Wks[:, dc * 256:(dc + 1) * 256],
                                  Wk_d[dc * 128:(dc + 1) * 128, :])
                nc.sync.dma_start(Wvs[:, dc * 256:(dc + 1) * 256],
                                  Wv_d[dc * 128:(dc + 1) * 128, :])
            cosQ = pa.tile([128, NQ], BF16, tag="cosq", name="cosq")
            nsinQ = pa.tile([128, NQ], BF16, tag="nsinq", name="nsinq")
            nc.sync.dma_start(cosQ[:], cosQ_d[:])
            nc.sync.dma_start(nsinQ[:], nsinQ_d[:])
            xq = []
            for dc in range(8):
                t = pa.tile([128, NQ], BF16, tag=f"xq{dc}", name=f"xq{dc}")
                nc.sync.dma_start(t[:], xqT_d[dc * 128:(dc + 1) * 128, :])
                xq.append(t)
            cosK = pa.tile([128, N], BF16, tag="cosk", name="cosk")
            nsinK = pa.tile([128, N], BF16, tag="nsink", name="nsink")
            nc.sync.dma_start(cosK[:], cosK_d[:])
            nc.sync.dma_start(nsinK[:], nsinK_d[:])

            def rope(psum, cos_s, nsin_s, w):
                """Return (t1, tmp): rope result = t1 + tmp (caller adds)."""
                tmp = pa.tile([128, w], F32, tag="rtmp", bufs=3, name="rtmp")
                for blk in range(4):
                    src = (blk ^ 1) * 32
                    nc.vector.tensor_copy(tmp[blk * 32:(blk + 1) * 32, :],
                                          psum[src:src + 32, :])
                t1 = pa.tile([128, w], F32, tag="rt1", bufs=3, name="rt1")
                nc.vector.tensor_mul(t1[:], psum[:], cos_s)
                nc.vector.tensor_mul(tmp[:], tmp[:], nsin_s)
                return t1, tmp

            # ---- Q^T projection + rope (only needs xqT/Wq) ----
            def q_proj(hp_range):
                for hp in hp_range:            # head pair (2hp, 2hp+1)
                    g = hp // 2
                    t = g // 2
                    base = (g % 2) * 64
                    hi = 2 * hp - 4 * g
                    wq = pa.tile([128, 8 * 128], BF16, tag="wq", bufs=2,
                                 name="wq")
                    for dc in range(8):
                        nc.sync.dma_start(
                            wq[:, dc * 128:(dc + 1) * 128],
                            Wq_d[dc * 128:(dc + 1) * 128,
                                 hp * 128:(hp + 1) * 128])
                    for nch in range(2):
                        pq = pap.tile([128, 512], F32, tag="mm", bufs=2,
                                      name="pq")
                        for dc in range(8):
                            nc.tensor.matmul(
                                pq[:], wq[:, dc * 128:(dc + 1) * 128],
                                xq[dc][:, nch * 512:(nch + 1) * 512],
                                start=(dc == 0), stop=(dc == 7))
                        c0 = nch * 512
                        t1, tmp = rope(pq, cosQ[:, c0:c0 + 512],
                                       nsinQ[:, c0:c0 + 512], 512)
                        d0 = hi * NQ + c0
                        d1 = (hi + 1) * NQ + c0
                        nc.vector.tensor_add(
                            Qrt[t][base:base + 64, d0:d0 + 512],
                            t1[0:64, :], tmp[0:64, :])
                        nc.vector.tensor_add(
                            Qrt[t][base:base + 64, d1:d1 + 512],
                            t1[64:128, :], tmp[64:128, :])

            # ---- K^T / V projections for one 512-row n-chunk ----
            def kv_proj(nch):          # nch in 0..7 (512 keys each)
                xt = []
                for dc in range(8):
                    t = pa.tile([128, 512], BF16, tag=f"xt{dc}", bufs=2,
                                name=f"xt{dc}")
                    nc.sync.dma_start(
                        t[:], xT_d[dc * 128:(dc + 1) * 128,
                                   nch * 512:(nch + 1) * 512])
                    xt.append(t)
                # V: 4 chunks of 128 rows
                for vc in range(4):
                    pv = pap.tile([128, 256], F32, tag="mm", bufs=2, name="pv")
                    for dc in range(8):
                        nc.tensor.matmul(
                            pv[:], xt[dc][:, vc * 128:(vc + 1) * 128],
                            Wvs[:, dc * 256:(dc + 1) * 256],
                            start=(dc == 0), stop=(dc == 7))
                    blk0 = (nch * 4 + vc) * HKV * VBLK
                    dst = Vsb[:, blk0:blk0 + HKV * VBLK].rearrange(
                        "p (g c) -> p g c", c=VBLK)[:, :, 0:64]
                    src = pv.rearrange("p (g c) -> p g c", c=64)
                    nc.vector.tensor_copy(dst, src)
                # K^T: one 512-wide chunk per pair tensor
                c0 = nch * 512
                for pt in range(2):
                    pk = pap.tile([128, 512], F32, tag="mm", bufs=2, name="pk")
                    for dc in range(8):
                        nc.tensor.matmul(
                            pk[:],
                            Wks[:, dc * 256 + pt * 128:
                                dc * 256 + (pt + 1) * 128],
                            xt[dc][:, 0:512],
                            start=(dc == 0), stop=(dc == 7))
                    t1, tmp = rope(pk, cosK[:, c0:c0 + 512],
                                   nsinK[:, c0:c0 + 512], 512)
                    nc.vector.tensor_add(Krt[pt][:, c0:c0 + 512],
                                         t1[:], tmp[:])

            # ---- attention pass over 16 key-chunks ----
            opart = {}

            def attn_pass(p):          # p = 0 or 1
                kcs = range(p * 16, (p + 1) * 16)
                for pt in range(2):
                    g0, g1 = 2 * pt, 2 * pt + 1
                    for hi in range(4):
                        for qh in range(2):
                            q0 = hi * NQ + qh * 512
                            oA = pbp.tile([65, 512], F32, tag="oA", bufs=1,
                                          name="oA")
                            oB = pbp.tile([65, 512], F32, tag="oB", bufs=1,
                                          name="oB")
                            if p == 1:
                                oiA = pb.tile([65, 512], F32, tag="oiA",
                                              bufs=2, name="oiA")
                                oiB = pb.tile([65, 512], F32, tag="oiB",
                                              bufs=2, name="oiB")
                                nc.sync.dma_start(
                                    oiA[:], opart[(pt, hi, qh, 0)][:])
                                nc.sync.dma_start(
                                    oiB[:], opart[(pt, hi, qh, 1)][:])
                            for kc in kcs:
                                st = pbp.tile([128, 1024], F32, tag="st",
                                              bufs=2, name="st")
                                nc.tensor.matmul(
                                    st[:, 0:512],
                                    Krt[pt][0:64, kc * 128:(kc + 1) * 128],
                                    Qrt[pt][0:64, q0:q0 + 512],
                                    start=True, stop=True)
                                nc.tensor.matmul(
                                    st[:, 512:1024],
                                    Krt[pt][64:128, kc * 128:(kc + 1) * 128],
                                    Qrt[pt][64:128, q0:q0 + 512],
                                    start=True, stop=True)
                                pT = pb.tile([128, 1024], BF16, tag="pT",
                                             bufs=3, name="pT")
                                nc.scalar.activation(pT[:], st[:], EXP,
                                                     scale=0.125)
                                vb0 = (kc * HKV + g0) * VBLK
                                vb1 = (kc * HKV + g1) * VBLK
                                nc.tensor.matmul(
                                    oA[:], Vsb[:, vb0:vb0 + VBLK],
                                    pT[:, 0:512],
                                    start=(kc == kcs[0]),
                                    stop=(kc == kcs[-1]))
                                nc.tensor.matmul(
                                    oB[:], Vsb[:, vb1:vb1 + VBLK],
                                    pT[:, 512:1024],
                                    start=(kc == kcs[0]),
                                    stop=(kc == kcs[-1]))
                            if p == 0:
                                # flush partial accumulators to DRAM
                                for i, o in enumerate((oA, oB)):
                                    osb = pb.tile([65, 512], F32, tag="osb",
                                                  bufs=2, name="osb")
                                    nc.vector.tensor_copy(osb[:], o[:])
                                    od = dp.tile([65, 512], F32,
                                                 tag=f"op{pt}{hi}{qh}{i}",
                                                 name=f"op{pt}{hi}{qh}{i}")
                                    opart[(pt, hi, qh, i)] = od
                                    nc.sync.dma_start(od[:], osb[:])
                            else:
                                # combine, normalize, store to Ot
                                for o, oi, g in ((oA, oiA, g0), (oB, oiB, g1)):
                                    head = 4 * g + hi
                                    hc, row = head // 2, (head % 2) * 64
                                    ts_ = pb.tile([65, 512], F32, tag="ts",
                                                  bufs=2, name="ts")
                                    nc.vector.tensor_add(ts_[:], o[:], oi[:])
                                    rinv = pb.tile([1, 512], F32, tag="rinv",
                                                   bufs=2, name="rinv")
                                    nc.vector.reciprocal(rinv[:],
                                                         ts_[64:65, :])
                                    sc = dp.tile([1, 512], F32, tag="rsc",
                                                 bufs=2, name="rsc")
                                    nc.sync.dma_start(sc[:], rinv[:])
                                    rb = pb.tile([64, 512], F32, tag="rb",
                                                 bufs=2, name="rb")
                                    nc.sync.dma_start(
                                        rb[:],
                                        bass.AP(sc.tensor, sc.offset,
                                                [[0, 64], [1, 512]]))
                                    nc.vector.tensor_mul(
                                        Ot[hc][row:row + 64,
                                               qh * 512:(qh + 1) * 512],
                                        ts_[0:64, :], rb[:])

            # ---- emission order drives scheduling priority ----
            q_proj(range(0, 4))        # unblocks pt=0 attention
            for nch in range(4):       # first half of K/V
                kv_proj(nch)
            q_proj(range(4, 8))
            attn_pass(0)               # overlaps with second half below
            for nch in range(4, 8):
                kv_proj(nch)
            for i in range(8):
                nc.sync.dma_start(Wo16[i][:], Wo_d[i * 128:(i + 1) * 128, :])
            attn_pass(1)

        # ---- phase C: output projection (attention pools released) ----
        with tc.tile_pool(name="pc", bufs=1) as pc, \
             tc.tile_pool(name="pcp", bufs=1, space="PSUM") as pcp:
            for qc in range(8):
                yp = pcp.tile([128, 1024], F32, tag="y", bufs=2, name="yp")
                for dh in range(2):
                    for hc in range(8):
                        nc.tensor.matmul(
                            yp[:, dh * 512:(dh + 1) * 512],
                            Ot[hc][:, qc * 128:(qc + 1) * 128],
                            Wo16[hc][:, dh * 512:(dh + 1) * 512],
                            start=(hc == 0), stop=(hc == 7))
                ys = pc.tile([128, 1024], F32, tag="ys", bufs=2, name="ys")
                nc.vector.tensor_copy(ys[:], yp[:])
                nc.sync.dma_start(out_d[qc * 128:(qc + 1) * 128, :], ys[:])

    nc.compile()
    return nc


def get_nc():
    if "nc" not in _CACHE:
        _CACHE["nc"] = _build()
    return _CACHE["nc"]


def _rope_tables():
    inv_freq = 1.0 / (10000.0 ** (np.arange(0, HD, 2, dtype=np.float32) / HD))
    t = np.arange(N, dtype=np.float32)
    freqs = np.outer(t, inv_freq)
    emb = np.concatenate([freqs, freqs], -1)        # [N, HD]
    return np.cos(emb).astype(np.float32), np.sin(emb).astype(np.float32)


def make_in_maps(x, Wq, Wk, Wv, Wo):
    cos, sin = _rope_tables()
    cosT = np.ascontiguousarray(cos.T)              # [64, N]
    nsinT = np.ascontiguousarray(sin.T)
    nsinT[0:32] = -nsinT[0:32]
    cosK = np.vstack([cosT, cosT])                  # [128, N]
    nsinK = np.vstack([nsinT, nsinT])

    bf = lambda a: np.ascontiguousarray(a).astype(bfloat16)
    Wq16, Wk16, Wv16, Wo16 = bf(Wq), bf(Wk), bf(Wv), bf(Wo)
    cosK16, nsinK16 = bf(cosK), bf(nsinK)

    in_maps = []
    for c in range(N_CORES):
        b, s = c // 4, c % 4
        xT = bf(x[b].T)                             # [D, N]
        xqT = bf(x[b, s * NQ:(s + 1) * NQ, :].T)    # [D, NQ]
        in_maps.append({
            "xT": xT, "xqT": xqT,
            "Wq": Wq16, "Wk": Wk16, "Wv": Wv16, "Wo": Wo16,
            "cosK": cosK16, "nsinK": nsinK16,
            "cosQ": bf(cosK[:, s * NQ:(s + 1) * NQ]),
            "nsinQ": bf(nsinK[:, s * NQ:(s + 1) * NQ]),
        })
    return in_maps


def assemble(results):
    out = np.zeros((B, N, D), np.float32)
    for c in range(N_CORES):
        b, s = c // 4, c % 4
        out[b, s * NQ:(s + 1) * NQ, :] = results[c]["out"]
    return out


def kernel(x, Wq, Wk, Wv, Wo):
    nc = get_nc()
    in_maps = make_in_maps(np.asarray(x, np.float32), np.asarray(Wq, np.float32),
                           np.asarray(Wk, np.float32), np.asarray(Wv, np.float32),
                           np.asarray(Wo, np.float32))
    res = bass_utils.run_bass_kernel_spmd(nc, in_maps,
                                          core_ids=list(range(N_CORES)))
    return assemble(res.results)
